# revision 18
# baseline (speedup 1.0000x reference)
"""Trainium2 Bass kernel for the DiCNN (WaveNet-like) module — v4.

Sharding: pure data parallelism - 4 batch items per core on 8 cores.
On-chip layout: channels on partitions, time on the free dim; the four
batch items are stacked as 4x32-partition bands (block-diag weights),
2x64 bands for the 64-channel input layer.

v4 design notes (vs the 106.6us v2/v3 baseline):
- Conv composition: pg0 = (wd0*wc) (*) x (3 taps) and
  pg1 = (wd1*wc) (*) x (4 taps) + (wd1*wo0) (*)_2 g0 (2 taps), with all
  biases folded in on the host.  This deletes z0/z1 and their PSUM->SBUF
  round-trips from the per-tile critical chain (z0-add -> pg0 and
  psB -> z1 -> pg1 were ~2.3us of serial latency per tile).
- PE HAM clock pinning: the PE defaults to K=4/8 (1.2 GHz) and only
  reaches 2.4 GHz after a ~3.4us fully-busy activity window; the trace
  showed the whole steady state ran at 1.2 GHz.  Dependency-free
  heartbeat matmuls (dedicated PSUM bank) fill every dependency stall
  so the array stays in the fast state.
- Output stores are 3584B-contiguous per partition: s1 is stored
  time-deinterleaved (phys col u*128+p holds t=4p+u) so each store
  partition covers 4 consecutive t rows (DMA descriptors 896B -> 3584B;
  store queue time ~62us -> ~41us aggregate).
- Output stored bf16; final bias b_sk2 and fp32 upcast happen on host.
"""

import numpy as np
import ml_dtypes

import concourse.bacc as bacc
import concourse.tile as tile
from concourse import mybir
from concourse.bass_utils import run_bass_kernel_spmd

BF16 = mybir.dt.bfloat16
FP32 = mybir.dt.float32

B, T, C_IN, HID, C_OUT, K = 32, 4096, 64, 32, 448, 2
N_CORES = 8
BPC = B // N_CORES          # batches per core = 4
TT = 512                    # time-tile size
NT = T // TT                # 8 tiles
XCOLS = 4100                # 3 zero lead cols + 4096 + 1 pad
DELTA = 2                   # output-stage pipeline delay in tiles
N_WARMUP = 9                # dependency-free warm-up matmuls
WARM_COLS = 384             # warm-up heartbeat width
HB_COLS = 256               # steady-state heartbeat width

AF = mybir.ActivationFunctionType
ALU = mybir.AluOpType

_cached_nc = None


def _f(x):
    return np.asarray(x, dtype=np.float32)


def _bf(x):
    return np.asarray(x, dtype=np.float32).astype(ml_dtypes.bfloat16)


def _tile4(v):
    return np.tile(_f(v).reshape(-1), 4)


def diag4(w32):
    s = np.zeros((128, 128), np.float32)
    for i in range(4):
        s[32 * i:32 * i + 32, 32 * i:32 * i + 32] = w32
    return s


def pairmat(w_32x64):
    """[32hid, 64in] -> two [128,128] block mats (one per batch pair)."""
    out = np.zeros((2, 128, 128), np.float32)
    wT = w_32x64.T  # [64, 32]
    for p in range(2):
        s = np.zeros((128, 128), np.float32)
        s[0:64, 64 * p:64 * p + 32] = wT
        s[64:128, 64 * p + 32:64 * p + 64] = wT
        out[p] = s
    return out


def prepare_weights(w_causal, b_causal, wd0, bd0, ws0, bs0, wo0, bo0,
                    wd1, bd1, ws1, bs1, wo1, bo1, w_sk1, b_sk1, w_sk2, b_sk2):
    """Host-side conv composition + weight layout (identical per core)."""
    del wo1, bo1  # dead code: z after the last block is never used

    Wc = [_f(w_causal)[:, :, k] for k in range(2)]      # [32, 64]
    Wd0 = [_f(wd0)[:, :, k] for k in range(2)]          # [32, 32]
    Wd1 = [_f(wd1)[:, :, k] for k in range(2)]
    Wo0 = _f(wo0)[:, :, 0]

    # pg0 taps on x[t-2+m], m in 0..2
    P = [Wd0[0] @ Wc[0],
         Wd0[0] @ Wc[1] + Wd0[1] @ Wc[0],
         Wd0[1] @ Wc[1]]
    # pg1 taps on x[t-3+m], m = 2k+j in 0..3
    Q = [Wd1[k] @ Wc[j] for k in range(2) for j in range(2)]
    # pg1 taps on g0[t-2+2k]
    R = [Wd1[0] @ Wo0, Wd1[1] @ Wo0]

    # x-tap weights: [128, 2 pairs * 7 taps, 128]; slot 7p+m: pg0 m=0..2,
    # pg1 m=3..6
    wx = np.zeros((128, 14, 128), np.float32)
    for p in range(2):
        for m in range(3):
            wx[:, 7 * p + m, :] = pairmat(P[m])[p]
        for m in range(4):
            wx[:, 7 * p + 3 + m, :] = pairmat(Q[m])[p]

    # diag4 32x32 mats: R0, R1, ws0, ws1, w_sk1
    wg = np.zeros((128, 5, 128), np.float32)
    wg[:, 0, :] = diag4(R[0].T)
    wg[:, 1, :] = diag4(R[1].T)
    wg[:, 2, :] = diag4(_f(ws0)[:, :, 0].T)
    wg[:, 3, :] = diag4(_f(ws1)[:, :, 0].T)
    wg[:, 4, :] = diag4(_f(w_sk1)[:, :, 0].T)

    # final conv weights, replicated on all four 32-partition bands
    w2r = np.zeros((128, 448), np.float32)
    w2T = _f(w_sk2)[:, :, 0].T
    for q in range(4):
        w2r[32 * q:32 * q + 32, :] = w2T

    bc = _f(b_causal)
    bg0 = (Wd0[0] + Wd0[1]) @ bc + _f(bd0)
    bg1 = (Wd1[0] + Wd1[1]) @ (bc + _f(bo0)) + _f(bd1)
    bvecs = np.zeros((128, 4), np.float32)
    bvecs[:, 0] = _tile4(bg0)
    bvecs[:, 1] = _tile4(bg1)
    bvecs[:, 2] = _tile4(_f(bs0) + _f(bs1))
    bvecs[:, 3] = _tile4(b_sk1)

    return dict(wx=_bf(wx), wg=_bf(wg), w2r=_bf(w2r),
                bvecs=np.ascontiguousarray(bvecs))


def prepare_x(x, core):
    """Per-core pre-transposed input staging array [2, 128, XCOLS] bf16.

    Cols 0..2 are the causal zero pad (t=-3..-1); col 3+t holds x[b,t,:]
    for the two batches of pair p stacked on the channel axis.
    """
    xT = np.zeros((2, 128, XCOLS), ml_dtypes.bfloat16)
    xb = _bf(x)
    for p in range(2):
        xT[p, 0:64, 3:3 + T] = xb[4 * core + 2 * p].T
        xT[p, 64:128, 3:3 + T] = xb[4 * core + 2 * p + 1].T
    return xT


def build_nc():
    nc = bacc.Bacc("TRN2", target_bir_lowering=False, debug=False,
                   num_devices=N_CORES)

    xT_d = nc.dram_tensor("xT", [2, 128, XCOLS], BF16, kind="ExternalInput")
    wx_d = nc.dram_tensor("wx", [128, 14, 128], BF16, kind="ExternalInput")
    wg_d = nc.dram_tensor("wg", [128, 5, 128], BF16, kind="ExternalInput")
    w2_d = nc.dram_tensor("w2r", [128, 448], BF16, kind="ExternalInput")
    bv_d = nc.dram_tensor("bvecs", [128, 4], FP32, kind="ExternalInput")
    y_d = nc.dram_tensor("y", [BPC, T, C_OUT], BF16, kind="ExternalOutput")

    with tile.TileContext(nc) as tc:
        with (
            tc.tile_pool(name="const", bufs=1) as const,
            tc.tile_pool(name="persist", bufs=1) as persist,
            tc.tile_pool(name="act", bufs=3) as actp,
            tc.tile_pool(name="gtile", bufs=2) as gtp,
            tc.tile_pool(name="stg", bufs=2) as stgp,
            tc.tile_pool(name="pgg", bufs=2, space="PSUM") as pggp,
            tc.tile_pool(name="pks", bufs=1, space="PSUM") as pksp,
            tc.tile_pool(name="pout", bufs=2, space="PSUM") as poutp,
            tc.tile_pool(name="phb", bufs=1, space="PSUM") as phbp,
        ):
            # ---- heartbeat operands first: memset needs no DMA, so the PE
            # can start warming the HAM clock before any input arrives ----
            hbw_s = const.tile([128, 128], BF16)
            nc.vector.memset(hbw_s[:], 0.0)
            wu_t = persist.tile([128, TT], BF16, tag="wu")
            nc.vector.memset(wu_t[:], 0.0)
            hb_ps = phbp.tile([128, TT], FP32, tag="phb", name="hb_ps")

            hb_cnt = [0]

            def heartbeat(n, cols=HB_COLS):
                """Dependency-free PE filler matmuls: keep the HAM activity
                window busy so the 2.4 GHz clock state is reached/held."""
                for _ in range(n):
                    hb_cnt[0] += 1
                    nc.tensor.matmul(hb_ps[:, 0:cols], hbw_s[:],
                                     wu_t[:, 0:cols], start=True, stop=True)

            # ---- constants (wx first - the warm-up burst needs it) ----
            wx_s = const.tile([128, 14, 128], BF16)
            nc.sync.dma_start(wx_s[:], wx_d.ap())
            # x chunk 0 next: it gates tile 0's first conv group
            x_s = [persist.tile([128, XCOLS], BF16, tag=f"x{p}",
                                name=f"x_s{p}") for p in range(2)]
            XCUTS = (0, 1028, 2052, 3076, XCOLS)
            for p in range(2):
                nc.sync.dma_start(x_s[p][:, 0:XCUTS[1]], xT_d[p, :, 0:XCUTS[1]])
            wg_s = const.tile([128, 5, 128], BF16)
            nc.sync.dma_start(wg_s[:], wg_d.ap())
            w2_s = const.tile([128, 448], BF16)
            nc.sync.dma_start(w2_s[:], w2_d.ap())
            bv_s = const.tile([128, 4], FP32)
            nc.sync.dma_start(bv_s[:], bv_d.ap())

            bg0_v = bv_s[:, 0:1]
            bg1_v = bv_s[:, 1:2]
            bskip_v = bv_s[:, 2:3]
            bsk1_v = bv_s[:, 3:4]

            # ---- persistent activations ----
            for a, b in zip(XCUTS[1:], XCUTS[2:]):
                for p in range(2):
                    nc.sync.dma_start(x_s[p][:, a:b], xT_d[p, :, a:b])
            g0_s = persist.tile([128, 4100], BF16, tag="g0")
            nc.vector.memset(g0_s[:, 0:2], 0.0)
            # s1 features, time-deinterleaved within each 512 tile:
            # phys col u*128+p holds t = 4p+u
            s1_s = persist.tile([128, T], BF16, tag="s1")

            # dummy sigmoid first: steer the ACT-table pass to the
            # sigmoid_and_others set (contains tanh/relu/copy too) so only
            # one ACT_TABLE_LOAD is emitted instead of two
            scr_t = const.tile([128, 8], FP32)
            nc.vector.memset(scr_t[:], 0.0)
            nc.scalar.activation(scr_t[:], scr_t[:], AF.Sigmoid)

            # ---- PE warm-up burst (overlaps the input DMAs) ----
            heartbeat(N_WARMUP, cols=WARM_COLS)

            evac_cnt = [0]

            def emit_out_group(it, j, h, po):
                """Final matmuls for chunk j, band pair h of tile it: the two
                bands as a row-tiled pack (distinct row groups -> concurrent
                PE sub-arrays, LDWEIGHTS overlaps in-flight matmuls)."""
                tc0 = TT * it + 128 * j
                for b in (2 * h, 2 * h + 1):
                    nc.tensor.matmul(po[:, b - 2 * h, 0:448],
                                     s1_s[32 * b:32 * b + 32, tc0:tc0 + 128],
                                     w2_s[32 * b:32 * b + 32, :],
                                     start=True, stop=True,
                                     tile_position=(32 * b, 0))

            def emit_evac(it, j, h, po, stg):
                # strict alternation: consecutive pout slots must evacuate
                # on different engines so the slot ping-pong overlaps
                if evac_cnt[0] % 2 == 1:
                    nc.scalar.copy(stg[:, 2 * h:2 * h + 2, j, :],
                                   po[:, :, 0:448])
                else:
                    nc.vector.tensor_copy(stg[:, 2 * h:2 * h + 2, j, :],
                                          po[:, :, 0:448])
                evac_cnt[0] += 1

            def emit_stores(it, stg, qs):
                t0 = TT * it
                for q in qs:
                    yap = y_d[q, t0:t0 + TT, :].rearrange(
                        "(p u) c -> p u c", p=128, u=4)
                    nc.sync.dma_start(yap, stg[:, q])

            def out_stage(it, phase, state):
                """Emit one phase (0..7) of tile `it`'s output stage."""
                if it < 0:
                    if phase % 2 == 0:
                        heartbeat(1)
                    return
                if phase == 0:
                    state[it] = stgp.tile([128, 4, 4, 448], BF16,
                                          tag="stg", name=f"stg_{it}")
                j, h = phase >> 1, phase & 1
                po = poutp.tile([128, 2, TT], FP32, tag="po",
                                name=f"po_{it}_{phase}")
                emit_out_group(it, j, h, po)
                emit_evac(it, j, h, po, state[it])
                if phase == 6:
                    emit_stores(it, state[it], (0, 1))
                elif phase == 7:
                    emit_stores(it, state[it], (2, 3))

            def weave(it, phase, state):
                out_stage(it - DELTA, phase, state)

            def emit_body(it, state):
                t0 = TT * it
                jt = it

                # -- pg0 = (wd0*wc) (*) x : 3 taps, col-tiled pairs
                pg0 = pggp.tile([128, TT], FP32, tag="pgg", name=f"pg0_{it}")
                for m in range(3):
                    for p in range(2):
                        rhs = x_s[p][:, t0 + 1 + m:t0 + 1 + m + TT]
                        nc.tensor.matmul(pg0[64 * p:64 * p + 64, :],
                                         wx_s[:, 7 * p + m, 64 * p:64 * p + 64],
                                         rhs, start=(m == 0), stop=(m == 2),
                                         tile_position=(0, 64 * p))
                weave(jt, 0, state)
                a0 = actp.tile([128, TT], BF16, tag="a", name=f"a0_{it}")
                nc.scalar.activation(a0[:], pg0[:], AF.Tanh, bias=bg0_v)
                b0 = actp.tile([128, TT], BF16, tag="b", name=f"b0_{it}")
                nc.scalar.activation(b0[:], pg0[:], AF.Sigmoid, bias=bg0_v)
                weave(jt, 1, state)

                # -- pg1 = (wd1*wc) (*) x (4 taps) + (wd1*wo0) (*)_2 g0;
                # x taps first so only the R taps wait on this tile's g0
                pg1 = pggp.tile([128, TT], FP32, tag="pgg", name=f"pg1_{it}")
                for m in range(4):
                    for p in range(2):
                        rhs = x_s[p][:, t0 + m:t0 + m + TT]
                        nc.tensor.matmul(pg1[64 * p:64 * p + 64, :],
                                         wx_s[:, 7 * p + 3 + m,
                                              64 * p:64 * p + 64],
                                         rhs, start=(m == 0), stop=False,
                                         tile_position=(0, 64 * p))
                heartbeat(2)
                # gated mul on the otherwise-idle GpSimd engine; the added
                # latency is absorbed (next tile's body is independent)
                nc.gpsimd.tensor_mul(g0_s[:, 2 + t0:2 + t0 + TT], a0[:], b0[:])
                weave(jt, 2, state)
                for k in range(2):
                    nc.tensor.matmul(pg1[:], wg_s[:, k, :],
                                     g0_s[:, t0 + 2 * k:t0 + 2 * k + TT],
                                     start=False, stop=(k == 1))
                # -- skip accum: ws0*g0 now, += ws1*g1 later
                psk = pksp.tile([128, TT], FP32, tag="pks", name=f"psk_{it}")
                nc.tensor.matmul(psk[:], wg_s[:, 2, :],
                                 g0_s[:, 2 + t0:2 + t0 + TT],
                                 start=True, stop=False)
                weave(jt, 3, state)
                a1 = actp.tile([128, TT], BF16, tag="a", name=f"a1_{it}")
                nc.scalar.activation(a1[:], pg1[:], AF.Tanh, bias=bg1_v)
                b1 = actp.tile([128, TT], BF16, tag="b", name=f"b1_{it}")
                nc.scalar.activation(b1[:], pg1[:], AF.Sigmoid, bias=bg1_v)
                weave(jt, 4, state)
                heartbeat(2)
                g1 = gtp.tile([128, TT], BF16, tag="g1", name=f"g1_{it}")
                nc.gpsimd.tensor_mul(g1[:], a1[:], b1[:])
                weave(jt, 5, state)

                # -- head: s0 = relu(skip + bias), s1 = relu(wsk1@s0 + bias)
                nc.tensor.matmul(psk[:], wg_s[:, 3, :], g1[:],
                                 start=False, stop=True)
                heartbeat(2)
                weave(jt, 6, state)
                s0_t = gtp.tile([128, TT], BF16, tag="s0", name=f"s0_{it}")
                # relu on DVE (ACT is the busier engine; tail slack absorbs
                # any queueing delay on this chain segment)
                nc.vector.tensor_scalar(s0_t[:], psk[:], bskip_v, 0.0,
                                        ALU.add, ALU.max)
                ps5 = pksp.tile([128, TT], FP32, tag="pks", name=f"ps5_{it}")
                # time-deinterleave via the moving operand: ps5 phys col
                # u*128+p gets t=4p+u, so the out-stage chunks produce
                # 4-consecutive-t store partitions at zero engine cost
                s0_deint = s0_t[:].rearrange("f (p u) -> f u p", p=128, u=4)
                nc.tensor.matmul(ps5[:], wg_s[:, 4, :], s0_deint,
                                 start=True, stop=True)
                heartbeat(2)
                weave(jt, 7, state)
                nc.vector.tensor_scalar(s1_s[:, t0:t0 + TT], ps5[:],
                                        bsk1_v, 0.0, ALU.add, ALU.max)

            state = {}
            for it in range(NT):
                emit_body(it, state)
            # trailing out-stages: interleave the remaining tiles' phases so
            # the two evac streams fill each other's pout ping-pong bubbles
            for phase in range(8):
                for itt in range(NT - DELTA, NT):
                    out_stage(itt, phase, state)
                    heartbeat(1)  # hold the clock through the drain

    nc.compile()
    return nc


def get_nc():
    global _cached_nc
    if _cached_nc is None:
        _cached_nc = build_nc()
    return _cached_nc


def kernel(**inputs):
    nc = get_nc()
    w = prepare_weights(
        inputs["w_causal"], inputs["b_causal"],
        inputs["wd0"], inputs["bd0"], inputs["ws0"], inputs["bs0"],
        inputs["wo0"], inputs["bo0"],
        inputs["wd1"], inputs["bd1"], inputs["ws1"], inputs["bs1"],
        inputs["wo1"], inputs["bo1"],
        inputs["w_sk1"], inputs["b_sk1"], inputs["w_sk2"], inputs["b_sk2"])
    x = np.asarray(inputs["x"])
    in_maps = [{"xT": prepare_x(x, c), **w} for c in range(N_CORES)]
    res = run_bass_kernel_spmd(nc, in_maps, list(range(N_CORES)))
    out = np.concatenate(
        [np.asarray(res.results[c]["y"]) for c in range(N_CORES)], axis=0)
    out = out.astype(np.float32) + _f(inputs["b_sk2"])[None, None, :]
    return out


# revision 20
# speedup vs baseline: 1.1797x; 1.1797x over previous
"""Trainium2 Bass kernel for the DiCNN (WaveNet-like) module — v4.

Sharding: pure data parallelism - 4 batch items per core on 8 cores.
On-chip layout: channels on partitions, time on the free dim; the four
batch items are stacked as 4x32-partition bands (block-diag weights),
2x64 bands for the 64-channel input layer.

v4 design notes (vs the 106.6us v2/v3 baseline):
- Conv composition: pg0 = (wd0*wc) (*) x (3 taps) and
  pg1 = (wd1*wc) (*) x (4 taps) + (wd1*wo0) (*)_2 g0 (2 taps), with all
  biases folded in on the host.  This deletes z0/z1 and their PSUM->SBUF
  round-trips from the per-tile critical chain (z0-add -> pg0 and
  psB -> z1 -> pg1 were ~2.3us of serial latency per tile).
- PE HAM clock pinning: the PE defaults to K=4/8 (1.2 GHz) and only
  reaches 2.4 GHz after a ~3.4us fully-busy activity window; the trace
  showed the whole steady state ran at 1.2 GHz.  Dependency-free
  heartbeat matmuls (dedicated PSUM bank) fill every dependency stall
  so the array stays in the fast state.
- Output stores are 3584B-contiguous per partition: s1 is stored
  time-deinterleaved (phys col u*128+p holds t=4p+u) so each store
  partition covers 4 consecutive t rows (DMA descriptors 896B -> 3584B;
  store queue time ~62us -> ~41us aggregate).
- Output stored bf16; final bias b_sk2 and fp32 upcast happen on host.
"""

import numpy as np
import ml_dtypes

import concourse.bacc as bacc
import concourse.tile as tile
from concourse import mybir
from concourse.bass_utils import run_bass_kernel_spmd

BF16 = mybir.dt.bfloat16
FP32 = mybir.dt.float32

B, T, C_IN, HID, C_OUT, K = 32, 4096, 64, 32, 448, 2
N_CORES = 8
BPC = B // N_CORES          # batches per core = 4
TT = 512                    # time-tile size
NT = T // TT                # 8 tiles
XCOLS = 4100                # 3 zero lead cols + 4096 + 1 pad
DELTA = 2                   # output-stage pipeline delay in tiles
N_WARMUP = 12               # dependency-free warm-up matmuls
HB_COLS = 256               # steady-state heartbeat width

AF = mybir.ActivationFunctionType
ALU = mybir.AluOpType

_cached_nc = None


def _f(x):
    return np.asarray(x, dtype=np.float32)


def _bf(x):
    return np.asarray(x, dtype=np.float32).astype(ml_dtypes.bfloat16)


def _tile4(v):
    return np.tile(_f(v).reshape(-1), 4)


def diag4(w32):
    s = np.zeros((128, 128), np.float32)
    for i in range(4):
        s[32 * i:32 * i + 32, 32 * i:32 * i + 32] = w32
    return s


def pairmat(w_32x64):
    """[32hid, 64in] -> two [128,128] block mats (one per batch pair)."""
    out = np.zeros((2, 128, 128), np.float32)
    wT = w_32x64.T  # [64, 32]
    for p in range(2):
        s = np.zeros((128, 128), np.float32)
        s[0:64, 64 * p:64 * p + 32] = wT
        s[64:128, 64 * p + 32:64 * p + 64] = wT
        out[p] = s
    return out


def prepare_weights(w_causal, b_causal, wd0, bd0, ws0, bs0, wo0, bo0,
                    wd1, bd1, ws1, bs1, wo1, bo1, w_sk1, b_sk1, w_sk2, b_sk2):
    """Host-side conv composition + weight layout (identical per core)."""
    del wo1, bo1  # dead code: z after the last block is never used

    Wc = [_f(w_causal)[:, :, k] for k in range(2)]      # [32, 64]
    Wd0 = [_f(wd0)[:, :, k] for k in range(2)]          # [32, 32]
    Wd1 = [_f(wd1)[:, :, k] for k in range(2)]
    Wo0 = _f(wo0)[:, :, 0]

    # pg0 taps on x[t-2+m], m in 0..2
    P = [Wd0[0] @ Wc[0],
         Wd0[0] @ Wc[1] + Wd0[1] @ Wc[0],
         Wd0[1] @ Wc[1]]
    # pg1 taps on x[t-3+m], m = 2k+j in 0..3
    Q = [Wd1[k] @ Wc[j] for k in range(2) for j in range(2)]
    # pg1 taps on g0[t-2+2k]
    R = [Wd1[0] @ Wo0, Wd1[1] @ Wo0]

    # x-tap weights: [128, 2 pairs * 7 taps, 128]; slot 7p+m: pg0 m=0..2,
    # pg1 m=3..6
    wx = np.zeros((128, 14, 128), np.float32)
    for p in range(2):
        for m in range(3):
            wx[:, 7 * p + m, :] = pairmat(P[m])[p]
        for m in range(4):
            wx[:, 7 * p + 3 + m, :] = pairmat(Q[m])[p]

    # diag4 32x32 mats: R0, R1, ws0, ws1, w_sk1
    wg = np.zeros((128, 5, 128), np.float32)
    wg[:, 0, :] = diag4(R[0].T)
    wg[:, 1, :] = diag4(R[1].T)
    wg[:, 2, :] = diag4(_f(ws0)[:, :, 0].T)
    wg[:, 3, :] = diag4(_f(ws1)[:, :, 0].T)
    wg[:, 4, :] = diag4(_f(w_sk1)[:, :, 0].T)

    # final conv weights, replicated on all four 32-partition bands
    w2r = np.zeros((128, 448), np.float32)
    w2T = _f(w_sk2)[:, :, 0].T
    for q in range(4):
        w2r[32 * q:32 * q + 32, :] = w2T

    bc = _f(b_causal)
    bg0 = (Wd0[0] + Wd0[1]) @ bc + _f(bd0)
    bg1 = (Wd1[0] + Wd1[1]) @ (bc + _f(bo0)) + _f(bd1)
    bvecs = np.zeros((128, 4), np.float32)
    bvecs[:, 0] = _tile4(bg0)
    bvecs[:, 1] = _tile4(bg1)
    bvecs[:, 2] = _tile4(_f(bs0) + _f(bs1))
    bvecs[:, 3] = _tile4(b_sk1)

    return dict(wx=_bf(wx), wg=_bf(wg), w2r=_bf(w2r),
                bvecs=np.ascontiguousarray(bvecs))


def prepare_x(x, core):
    """Per-core pre-transposed input staging array [2, 128, XCOLS] bf16.

    Cols 0..2 are the causal zero pad (t=-3..-1); col 3+t holds x[b,t,:]
    for the two batches of pair p stacked on the channel axis.
    """
    xT = np.zeros((2, 128, XCOLS), ml_dtypes.bfloat16)
    xb = _bf(x)
    for p in range(2):
        xT[p, 0:64, 3:3 + T] = xb[4 * core + 2 * p].T
        xT[p, 64:128, 3:3 + T] = xb[4 * core + 2 * p + 1].T
    return xT


def build_nc():
    nc = bacc.Bacc("TRN2", target_bir_lowering=False, debug=False,
                   num_devices=N_CORES)

    xT_d = nc.dram_tensor("xT", [2, 128, XCOLS], BF16, kind="ExternalInput")
    wx_d = nc.dram_tensor("wx", [128, 14, 128], BF16, kind="ExternalInput")
    wg_d = nc.dram_tensor("wg", [128, 5, 128], BF16, kind="ExternalInput")
    w2_d = nc.dram_tensor("w2r", [128, 448], BF16, kind="ExternalInput")
    bv_d = nc.dram_tensor("bvecs", [128, 4], FP32, kind="ExternalInput")
    y_d = nc.dram_tensor("y", [BPC, T, C_OUT], BF16, kind="ExternalOutput")

    with tile.TileContext(nc) as tc:
        with (
            tc.tile_pool(name="const", bufs=1) as const,
            tc.tile_pool(name="persist", bufs=1) as persist,
            tc.tile_pool(name="act", bufs=3) as actp,
            tc.tile_pool(name="gtile", bufs=2) as gtp,
            tc.tile_pool(name="stg", bufs=2) as stgp,
            tc.tile_pool(name="pgg", bufs=2, space="PSUM") as pggp,
            tc.tile_pool(name="pks", bufs=1, space="PSUM") as pksp,
            tc.tile_pool(name="pout", bufs=2, space="PSUM") as poutp,
            tc.tile_pool(name="phb", bufs=1, space="PSUM") as phbp,
        ):
            # ---- heartbeat operands first: memset needs no DMA, so the PE
            # can start warming the HAM clock before any input arrives ----
            hbw_s = const.tile([128, 128], BF16)
            nc.vector.memset(hbw_s[:], 0.0)
            wu_t = persist.tile([128, TT], BF16, tag="wu")
            nc.vector.memset(wu_t[:], 0.0)
            hb_ps = phbp.tile([128, TT], FP32, tag="phb", name="hb_ps")

            hb_cnt = [0]

            def heartbeat(n, cols=HB_COLS):
                """Dependency-free PE filler matmuls: keep the HAM activity
                window busy so the 2.4 GHz clock state is reached/held."""
                for _ in range(n):
                    hb_cnt[0] += 1
                    nc.tensor.matmul(hb_ps[:, 0:cols], hbw_s[:],
                                     wu_t[:, 0:cols], start=True, stop=True)

            # ---- constants (wx first - the warm-up burst needs it) ----
            wx_s = const.tile([128, 14, 128], BF16)
            nc.sync.dma_start(wx_s[:], wx_d.ap())
            # x chunk 0 next: it gates tile 0's first conv group
            x_s = [persist.tile([128, XCOLS], BF16, tag=f"x{p}",
                                name=f"x_s{p}") for p in range(2)]
            XCUTS = (0, 1028, 2052, 3076, XCOLS)
            for p in range(2):
                nc.sync.dma_start(x_s[p][:, 0:XCUTS[1]], xT_d[p, :, 0:XCUTS[1]])
            wg_s = const.tile([128, 5, 128], BF16)
            nc.sync.dma_start(wg_s[:], wg_d.ap())
            w2_s = const.tile([128, 448], BF16)
            nc.sync.dma_start(w2_s[:], w2_d.ap())
            bv_s = const.tile([128, 4], FP32)
            nc.sync.dma_start(bv_s[:], bv_d.ap())

            bg0_v = bv_s[:, 0:1]
            bg1_v = bv_s[:, 1:2]
            bskip_v = bv_s[:, 2:3]
            bsk1_v = bv_s[:, 3:4]

            # ---- persistent activations ----
            for a, b in zip(XCUTS[1:], XCUTS[2:]):
                for p in range(2):
                    nc.sync.dma_start(x_s[p][:, a:b], xT_d[p, :, a:b])
            g0_s = persist.tile([128, 4100], BF16, tag="g0")
            nc.vector.memset(g0_s[:, 0:2], 0.0)
            # s1 features, time-deinterleaved within each 512 tile:
            # phys col u*128+p holds t = 4p+u
            s1_s = persist.tile([128, T], BF16, tag="s1")

            # dummy sigmoid first: steer the ACT-table pass to the
            # sigmoid_and_others set (contains tanh/relu/copy too) so only
            # one ACT_TABLE_LOAD is emitted instead of two
            scr_t = const.tile([128, 8], FP32)
            nc.vector.memset(scr_t[:], 0.0)
            nc.scalar.activation(scr_t[:], scr_t[:], AF.Sigmoid)

            # ---- PE warm-up burst (overlaps the input DMAs) ----
            heartbeat(N_WARMUP, cols=TT)

            evac_cnt = [0]

            def emit_out_group(it, j, h, po):
                """Final matmuls for chunk j, band pair h of tile it: the two
                bands as a row-tiled pack (distinct row groups -> concurrent
                PE sub-arrays, LDWEIGHTS overlaps in-flight matmuls)."""
                tc0 = TT * it + 128 * j
                for b in (2 * h, 2 * h + 1):
                    nc.tensor.matmul(po[:, b - 2 * h, 0:448],
                                     s1_s[32 * b:32 * b + 32, tc0:tc0 + 128],
                                     w2_s[32 * b:32 * b + 32, :],
                                     start=True, stop=True,
                                     tile_position=(32 * b, 0))

            def emit_evac(it, j, h, po, stg):
                # strict alternation: consecutive pout slots must evacuate
                # on different engines so the slot ping-pong overlaps
                if evac_cnt[0] % 2 == 1:
                    nc.scalar.copy(stg[:, 2 * h:2 * h + 2, j, :],
                                   po[:, :, 0:448])
                else:
                    nc.vector.tensor_copy(stg[:, 2 * h:2 * h + 2, j, :],
                                          po[:, :, 0:448])
                evac_cnt[0] += 1

            def emit_stores(it, stg, qs):
                t0 = TT * it
                for q in qs:
                    yap = y_d[q, t0:t0 + TT, :].rearrange(
                        "(p u) c -> p u c", p=128, u=4)
                    nc.sync.dma_start(yap, stg[:, q])

            def out_stage(it, phase, state):
                """Emit one phase (0..7) of tile `it`'s output stage."""
                if it < 0:
                    if phase % 2 == 0:
                        heartbeat(1)
                    return
                if phase == 0:
                    state[it] = stgp.tile([128, 4, 4, 448], BF16,
                                          tag="stg", name=f"stg_{it}")
                j, h = phase >> 1, phase & 1
                po = poutp.tile([128, 2, TT], FP32, tag="po",
                                name=f"po_{it}_{phase}")
                emit_out_group(it, j, h, po)
                emit_evac(it, j, h, po, state[it])
                if phase == 6:
                    emit_stores(it, state[it], (0, 1))
                elif phase == 7:
                    emit_stores(it, state[it], (2, 3))

            def weave(it, phase, state):
                out_stage(it - DELTA, phase, state)

            def emit_body(it, state):
                t0 = TT * it
                jt = it

                # -- pg0 = (wd0*wc) (*) x : 3 taps, col-tiled pairs
                pg0 = pggp.tile([128, TT], FP32, tag="pgg", name=f"pg0_{it}")
                for m in range(3):
                    for p in range(2):
                        rhs = x_s[p][:, t0 + 1 + m:t0 + 1 + m + TT]
                        nc.tensor.matmul(pg0[64 * p:64 * p + 64, :],
                                         wx_s[:, 7 * p + m, 64 * p:64 * p + 64],
                                         rhs, start=(m == 0), stop=(m == 2),
                                         tile_position=(0, 64 * p))
                weave(jt, 0, state)
                a0 = actp.tile([128, TT], BF16, tag="a", name=f"a0_{it}")
                nc.scalar.activation(a0[:], pg0[:], AF.Tanh, bias=bg0_v)
                b0 = actp.tile([128, TT], BF16, tag="b", name=f"b0_{it}")
                nc.scalar.activation(b0[:], pg0[:], AF.Sigmoid, bias=bg0_v)
                weave(jt, 1, state)

                # -- pg1 = (wd1*wc) (*) x (4 taps) + (wd1*wo0) (*)_2 g0;
                # x taps first so only the R taps wait on this tile's g0
                pg1 = pggp.tile([128, TT], FP32, tag="pgg", name=f"pg1_{it}")
                for m in range(4):
                    for p in range(2):
                        rhs = x_s[p][:, t0 + m:t0 + m + TT]
                        nc.tensor.matmul(pg1[64 * p:64 * p + 64, :],
                                         wx_s[:, 7 * p + 3 + m,
                                              64 * p:64 * p + 64],
                                         rhs, start=(m == 0), stop=False,
                                         tile_position=(0, 64 * p))
                heartbeat(2)
                nc.vector.tensor_mul(g0_s[:, 2 + t0:2 + t0 + TT], a0[:], b0[:])
                weave(jt, 2, state)
                for k in range(2):
                    nc.tensor.matmul(pg1[:], wg_s[:, k, :],
                                     g0_s[:, t0 + 2 * k:t0 + 2 * k + TT],
                                     start=False, stop=(k == 1))
                # -- skip accum: ws0*g0 now, += ws1*g1 later
                psk = pksp.tile([128, TT], FP32, tag="pks", name=f"psk_{it}")
                nc.tensor.matmul(psk[:], wg_s[:, 2, :],
                                 g0_s[:, 2 + t0:2 + t0 + TT],
                                 start=True, stop=False)
                weave(jt, 3, state)
                a1 = actp.tile([128, TT], BF16, tag="a", name=f"a1_{it}")
                nc.scalar.activation(a1[:], pg1[:], AF.Tanh, bias=bg1_v)
                b1 = actp.tile([128, TT], BF16, tag="b", name=f"b1_{it}")
                nc.scalar.activation(b1[:], pg1[:], AF.Sigmoid, bias=bg1_v)
                weave(jt, 4, state)
                heartbeat(2)
                g1 = gtp.tile([128, TT], BF16, tag="g1", name=f"g1_{it}")
                nc.vector.tensor_mul(g1[:], a1[:], b1[:])
                weave(jt, 5, state)

                # -- head: s0 = relu(skip + bias), s1 = relu(wsk1@s0 + bias)
                nc.tensor.matmul(psk[:], wg_s[:, 3, :], g1[:],
                                 start=False, stop=True)
                heartbeat(2)
                weave(jt, 6, state)
                s0_t = gtp.tile([128, TT], BF16, tag="s0", name=f"s0_{it}")
                # relu on DVE: ACT is the busier engine (4 transcendentals +
                # 4 evac copies/tile); the chain delay lands in DELTA slack
                nc.vector.tensor_scalar(s0_t[:], psk[:], bskip_v, 0.0,
                                        ALU.add, ALU.max)
                ps5 = pksp.tile([128, TT], FP32, tag="pks", name=f"ps5_{it}")
                # time-deinterleave via the moving operand: ps5 phys col
                # u*128+p gets t=4p+u, so the out-stage chunks produce
                # 4-consecutive-t store partitions at zero engine cost
                s0_deint = s0_t[:].rearrange("f (p u) -> f u p", p=128, u=4)
                nc.tensor.matmul(ps5[:], wg_s[:, 4, :], s0_deint,
                                 start=True, stop=True)
                heartbeat(2)
                weave(jt, 7, state)
                nc.vector.tensor_scalar(s1_s[:, t0:t0 + TT], ps5[:],
                                        bsk1_v, 0.0, ALU.add, ALU.max)

            state = {}
            for it in range(NT):
                emit_body(it, state)
            for itt in range(NT - DELTA, NT):
                for phase in range(8):
                    out_stage(itt, phase, state)

    nc.compile()
    return nc


def get_nc():
    global _cached_nc
    if _cached_nc is None:
        _cached_nc = build_nc()
    return _cached_nc


def kernel(**inputs):
    nc = get_nc()
    w = prepare_weights(
        inputs["w_causal"], inputs["b_causal"],
        inputs["wd0"], inputs["bd0"], inputs["ws0"], inputs["bs0"],
        inputs["wo0"], inputs["bo0"],
        inputs["wd1"], inputs["bd1"], inputs["ws1"], inputs["bs1"],
        inputs["wo1"], inputs["bo1"],
        inputs["w_sk1"], inputs["b_sk1"], inputs["w_sk2"], inputs["b_sk2"])
    x = np.asarray(inputs["x"])
    in_maps = [{"xT": prepare_x(x, c), **w} for c in range(N_CORES)]
    res = run_bass_kernel_spmd(nc, in_maps, list(range(N_CORES)))
    out = np.concatenate(
        [np.asarray(res.results[c]["y"]) for c in range(N_CORES)], axis=0)
    out = out.astype(np.float32) + _f(inputs["b_sk2"])[None, None, :]
    return out


# revision 22
# speedup vs baseline: 1.1936x; 1.0118x over previous
"""Trainium2 Bass kernel for the DiCNN (WaveNet-like) module — v4.

Sharding: pure data parallelism - 4 batch items per core on 8 cores.
On-chip layout: channels on partitions, time on the free dim; the four
batch items are stacked as 4x32-partition bands (block-diag weights),
2x64 bands for the 64-channel input layer.

v4 design notes (vs the 106.6us v2/v3 baseline):
- Conv composition: pg0 = (wd0*wc) (*) x (3 taps) and
  pg1 = (wd1*wc) (*) x (4 taps) + (wd1*wo0) (*)_2 g0 (2 taps), with all
  biases folded in on the host.  This deletes z0/z1 and their PSUM->SBUF
  round-trips from the per-tile critical chain (z0-add -> pg0 and
  psB -> z1 -> pg1 were ~2.3us of serial latency per tile).
- PE HAM clock pinning: the PE defaults to K=4/8 (1.2 GHz) and only
  reaches 2.4 GHz after a ~3.4us fully-busy activity window; the trace
  showed the whole steady state ran at 1.2 GHz.  Dependency-free
  heartbeat matmuls (dedicated PSUM bank) fill every dependency stall
  so the array stays in the fast state.
- Output stores are 3584B-contiguous per partition: s1 is stored
  time-deinterleaved (phys col u*128+p holds t=4p+u) so each store
  partition covers 4 consecutive t rows (DMA descriptors 896B -> 3584B;
  store queue time ~62us -> ~41us aggregate).
- Output stored bf16; final bias b_sk2 and fp32 upcast happen on host.
"""

import numpy as np
import ml_dtypes

import concourse.bacc as bacc
import concourse.tile as tile
from concourse import mybir
from concourse.bass_utils import run_bass_kernel_spmd

BF16 = mybir.dt.bfloat16
FP32 = mybir.dt.float32

B, T, C_IN, HID, C_OUT, K = 32, 4096, 64, 32, 448, 2
N_CORES = 8
BPC = B // N_CORES          # batches per core = 4
TT = 512                    # time-tile size
NT = T // TT                # 8 tiles
XCOLS = 4100                # 3 zero lead cols + 4096 + 1 pad
DELTA = 2                   # output-stage pipeline delay in tiles
N_WARMUP = 12               # dependency-free warm-up matmuls
HB_COLS = 256               # steady-state heartbeat width

AF = mybir.ActivationFunctionType
ALU = mybir.AluOpType

_cached_nc = None


def _f(x):
    return np.asarray(x, dtype=np.float32)


def _bf(x):
    return np.asarray(x, dtype=np.float32).astype(ml_dtypes.bfloat16)


def _tile4(v):
    return np.tile(_f(v).reshape(-1), 4)


def diag4(w32):
    s = np.zeros((128, 128), np.float32)
    for i in range(4):
        s[32 * i:32 * i + 32, 32 * i:32 * i + 32] = w32
    return s


def pairmat(w_32x64):
    """[32hid, 64in] -> two [128,128] block mats (one per batch pair)."""
    out = np.zeros((2, 128, 128), np.float32)
    wT = w_32x64.T  # [64, 32]
    for p in range(2):
        s = np.zeros((128, 128), np.float32)
        s[0:64, 64 * p:64 * p + 32] = wT
        s[64:128, 64 * p + 32:64 * p + 64] = wT
        out[p] = s
    return out


def prepare_weights(w_causal, b_causal, wd0, bd0, ws0, bs0, wo0, bo0,
                    wd1, bd1, ws1, bs1, wo1, bo1, w_sk1, b_sk1, w_sk2, b_sk2):
    """Host-side conv composition + weight layout (identical per core)."""
    del wo1, bo1  # dead code: z after the last block is never used

    Wc = [_f(w_causal)[:, :, k] for k in range(2)]      # [32, 64]
    Wd0 = [_f(wd0)[:, :, k] for k in range(2)]          # [32, 32]
    Wd1 = [_f(wd1)[:, :, k] for k in range(2)]
    Wo0 = _f(wo0)[:, :, 0]

    # pg0 taps on x[t-2+m], m in 0..2
    P = [Wd0[0] @ Wc[0],
         Wd0[0] @ Wc[1] + Wd0[1] @ Wc[0],
         Wd0[1] @ Wc[1]]
    # pg1 taps on x[t-3+m], m = 2k+j in 0..3
    Q = [Wd1[k] @ Wc[j] for k in range(2) for j in range(2)]
    # pg1 taps on g0[t-2+2k]
    R = [Wd1[0] @ Wo0, Wd1[1] @ Wo0]

    # x-tap weights: [128, 2 pairs * 7 taps, 128]; slot 7p+m: pg0 m=0..2,
    # pg1 m=3..6
    wx = np.zeros((128, 14, 128), np.float32)
    for p in range(2):
        for m in range(3):
            wx[:, 7 * p + m, :] = pairmat(P[m])[p]
        for m in range(4):
            wx[:, 7 * p + 3 + m, :] = pairmat(Q[m])[p]

    # diag4 32x32 mats: R0, R1, ws0, ws1, w_sk1
    wg = np.zeros((128, 5, 128), np.float32)
    wg[:, 0, :] = diag4(R[0].T)
    wg[:, 1, :] = diag4(R[1].T)
    wg[:, 2, :] = diag4(_f(ws0)[:, :, 0].T)
    wg[:, 3, :] = diag4(_f(ws1)[:, :, 0].T)
    wg[:, 4, :] = diag4(_f(w_sk1)[:, :, 0].T)

    # final conv weights, replicated on all four 32-partition bands
    w2r = np.zeros((128, 448), np.float32)
    w2T = _f(w_sk2)[:, :, 0].T
    for q in range(4):
        w2r[32 * q:32 * q + 32, :] = w2T

    bc = _f(b_causal)
    bg0 = (Wd0[0] + Wd0[1]) @ bc + _f(bd0)
    bg1 = (Wd1[0] + Wd1[1]) @ (bc + _f(bo0)) + _f(bd1)
    bvecs = np.zeros((128, 4), np.float32)
    bvecs[:, 0] = _tile4(bg0)
    bvecs[:, 1] = _tile4(bg1)
    bvecs[:, 2] = _tile4(_f(bs0) + _f(bs1))
    bvecs[:, 3] = _tile4(b_sk1)

    return dict(wx=_bf(wx), wg=_bf(wg), w2r=_bf(w2r),
                bvecs=np.ascontiguousarray(bvecs))


def prepare_x(x, core):
    """Per-core pre-transposed input staging array [2, 128, XCOLS] bf16.

    Cols 0..2 are the causal zero pad (t=-3..-1); col 3+t holds x[b,t,:]
    for the two batches of pair p stacked on the channel axis.
    """
    xT = np.zeros((2, 128, XCOLS), ml_dtypes.bfloat16)
    xb = _bf(x)
    for p in range(2):
        xT[p, 0:64, 3:3 + T] = xb[4 * core + 2 * p].T
        xT[p, 64:128, 3:3 + T] = xb[4 * core + 2 * p + 1].T
    return xT


def build_nc():
    nc = bacc.Bacc("TRN2", target_bir_lowering=False, debug=False,
                   num_devices=N_CORES)

    xT_d = nc.dram_tensor("xT", [2, 128, XCOLS], BF16, kind="ExternalInput")
    wx_d = nc.dram_tensor("wx", [128, 14, 128], BF16, kind="ExternalInput")
    wg_d = nc.dram_tensor("wg", [128, 5, 128], BF16, kind="ExternalInput")
    w2_d = nc.dram_tensor("w2r", [128, 448], BF16, kind="ExternalInput")
    bv_d = nc.dram_tensor("bvecs", [128, 4], FP32, kind="ExternalInput")
    y_d = nc.dram_tensor("y", [BPC, T, C_OUT], BF16, kind="ExternalOutput")

    with tile.TileContext(nc) as tc:
        with (
            tc.tile_pool(name="const", bufs=1) as const,
            tc.tile_pool(name="persist", bufs=1) as persist,
            tc.tile_pool(name="act", bufs=3) as actp,
            tc.tile_pool(name="gtile", bufs=2) as gtp,
            tc.tile_pool(name="stg", bufs=2) as stgp,
            tc.tile_pool(name="pgg", bufs=2, space="PSUM") as pggp,
            tc.tile_pool(name="pks", bufs=1, space="PSUM") as pksp,
            tc.tile_pool(name="pout", bufs=2, space="PSUM") as poutp,
            tc.tile_pool(name="phb", bufs=1, space="PSUM") as phbp,
        ):
            # ---- heartbeat operands first: memset needs no DMA, so the PE
            # can start warming the HAM clock before any input arrives ----
            hbw_s = const.tile([128, 128], BF16)
            nc.vector.memset(hbw_s[:], 0.0)
            wu_t = persist.tile([128, TT], BF16, tag="wu")
            nc.vector.memset(wu_t[:], 0.0)
            hb_ps = phbp.tile([128, TT], FP32, tag="phb", name="hb_ps")

            hb_cnt = [0]

            def heartbeat(n, cols=HB_COLS):
                """Dependency-free PE filler matmuls: keep the HAM activity
                window busy so the 2.4 GHz clock state is reached/held."""
                for _ in range(n):
                    hb_cnt[0] += 1
                    nc.tensor.matmul(hb_ps[:, 0:cols], hbw_s[:],
                                     wu_t[:, 0:cols], start=True, stop=True)

            # ---- constants (wx first - the warm-up burst needs it) ----
            wx_s = const.tile([128, 14, 128], BF16)
            nc.sync.dma_start(wx_s[:], wx_d.ap())
            # x chunk 0 next: it gates tile 0's first conv group
            x_s = [persist.tile([128, XCOLS], BF16, tag=f"x{p}",
                                name=f"x_s{p}") for p in range(2)]
            XCUTS = (0, 1028, 2052, 3076, XCOLS)
            for p in range(2):
                nc.sync.dma_start(x_s[p][:, 0:XCUTS[1]], xT_d[p, :, 0:XCUTS[1]])
            wg_s = const.tile([128, 5, 128], BF16)
            nc.sync.dma_start(wg_s[:], wg_d.ap())
            w2_s = const.tile([128, 448], BF16)
            nc.sync.dma_start(w2_s[:], w2_d.ap())
            bv_s = const.tile([128, 4], FP32)
            nc.sync.dma_start(bv_s[:], bv_d.ap())

            bg0_v = bv_s[:, 0:1]
            bg1_v = bv_s[:, 1:2]
            bskip_v = bv_s[:, 2:3]
            bsk1_v = bv_s[:, 3:4]

            # ---- persistent activations ----
            for a, b in zip(XCUTS[1:], XCUTS[2:]):
                for p in range(2):
                    nc.sync.dma_start(x_s[p][:, a:b], xT_d[p, :, a:b])
            g0_s = persist.tile([128, 4100], BF16, tag="g0")
            nc.vector.memset(g0_s[:, 0:2], 0.0)
            # s1 features, time-deinterleaved within each 512 tile:
            # phys col u*128+p holds t = 4p+u
            s1_s = persist.tile([128, T], BF16, tag="s1")

            # dummy sigmoid first: steer the ACT-table pass to the
            # sigmoid_and_others set (contains tanh/relu/copy too) so only
            # one ACT_TABLE_LOAD is emitted instead of two
            scr_t = const.tile([128, 8], FP32)
            nc.vector.memset(scr_t[:], 0.0)
            nc.scalar.activation(scr_t[:], scr_t[:], AF.Sigmoid)

            # ---- PE warm-up burst (overlaps the input DMAs) ----
            heartbeat(N_WARMUP, cols=TT)

            evac_cnt = [0]

            def emit_out_group(it, j, h, po):
                """Final matmuls for chunk j, band pair h of tile it: the two
                bands as a row-tiled pack (distinct row groups -> concurrent
                PE sub-arrays, LDWEIGHTS overlaps in-flight matmuls)."""
                tc0 = TT * it + 128 * j
                for b in (2 * h, 2 * h + 1):
                    nc.tensor.matmul(po[:, b - 2 * h, 0:448],
                                     s1_s[32 * b:32 * b + 32, tc0:tc0 + 128],
                                     w2_s[32 * b:32 * b + 32, :],
                                     start=True, stop=True,
                                     tile_position=(32 * b, 0))

            def emit_evac(it, j, h, po, stg):
                # strict alternation: consecutive pout slots must evacuate
                # on different engines so the slot ping-pong overlaps
                if evac_cnt[0] % 2 == 1:
                    nc.scalar.copy(stg[:, 2 * h:2 * h + 2, j, :],
                                   po[:, :, 0:448])
                else:
                    nc.vector.tensor_copy(stg[:, 2 * h:2 * h + 2, j, :],
                                          po[:, :, 0:448])
                evac_cnt[0] += 1

            def emit_stores(it, stg, qs):
                t0 = TT * it
                for q in qs:
                    yap = y_d[q, t0:t0 + TT, :].rearrange(
                        "(p u) c -> p u c", p=128, u=4)
                    nc.sync.dma_start(yap, stg[:, q])

            def out_stage(it, phase, state):
                """Emit one phase (0..7) of tile `it`'s output stage."""
                if it < 0:
                    if phase % 2 == 0:
                        heartbeat(1)
                    return
                if phase == 0:
                    state[it] = stgp.tile([128, 4, 4, 448], BF16,
                                          tag="stg", name=f"stg_{it}")
                j, h = phase >> 1, phase & 1
                po = poutp.tile([128, 2, TT], FP32, tag="po",
                                name=f"po_{it}_{phase}")
                emit_out_group(it, j, h, po)
                emit_evac(it, j, h, po, state[it])
                if phase == 6:
                    emit_stores(it, state[it], (0, 1))
                elif phase == 7:
                    emit_stores(it, state[it], (2, 3))

            def weave(it, slot, state):
                # half-shifted schedule: body(i) carries the BACK half of
                # out(i-2) then the FRONT half of out(i-1).  s1(i-1) gets a
                # half-tile of slack to clear the DVE queue before its out
                # stage starts, and only 1.5 out-stages trail the last body.
                if slot < 4:
                    out_stage(it - 2, 4 + slot, state)
                else:
                    out_stage(it - 1, slot - 4, state)

            def emit_body(it, state):
                t0 = TT * it
                jt = it

                # -- pg0 = (wd0*wc) (*) x : 3 taps, col-tiled pairs
                pg0 = pggp.tile([128, TT], FP32, tag="pgg", name=f"pg0_{it}")
                for m in range(3):
                    for p in range(2):
                        rhs = x_s[p][:, t0 + 1 + m:t0 + 1 + m + TT]
                        nc.tensor.matmul(pg0[64 * p:64 * p + 64, :],
                                         wx_s[:, 7 * p + m, 64 * p:64 * p + 64],
                                         rhs, start=(m == 0), stop=(m == 2),
                                         tile_position=(0, 64 * p))
                weave(jt, 0, state)
                a0 = actp.tile([128, TT], BF16, tag="a", name=f"a0_{it}")
                nc.scalar.activation(a0[:], pg0[:], AF.Tanh, bias=bg0_v)
                b0 = actp.tile([128, TT], BF16, tag="b", name=f"b0_{it}")
                nc.scalar.activation(b0[:], pg0[:], AF.Sigmoid, bias=bg0_v)
                weave(jt, 1, state)

                # -- pg1 = (wd1*wc) (*) x (4 taps) + (wd1*wo0) (*)_2 g0;
                # x taps first so only the R taps wait on this tile's g0
                pg1 = pggp.tile([128, TT], FP32, tag="pgg", name=f"pg1_{it}")
                for m in range(4):
                    for p in range(2):
                        rhs = x_s[p][:, t0 + m:t0 + m + TT]
                        nc.tensor.matmul(pg1[64 * p:64 * p + 64, :],
                                         wx_s[:, 7 * p + 3 + m,
                                              64 * p:64 * p + 64],
                                         rhs, start=(m == 0), stop=False,
                                         tile_position=(0, 64 * p))
                heartbeat(2)
                nc.vector.tensor_mul(g0_s[:, 2 + t0:2 + t0 + TT], a0[:], b0[:])
                weave(jt, 2, state)
                for k in range(2):
                    nc.tensor.matmul(pg1[:], wg_s[:, k, :],
                                     g0_s[:, t0 + 2 * k:t0 + 2 * k + TT],
                                     start=False, stop=(k == 1))
                # -- skip accum: ws0*g0 now, += ws1*g1 later
                psk = pksp.tile([128, TT], FP32, tag="pks", name=f"psk_{it}")
                nc.tensor.matmul(psk[:], wg_s[:, 2, :],
                                 g0_s[:, 2 + t0:2 + t0 + TT],
                                 start=True, stop=False)
                weave(jt, 3, state)
                a1 = actp.tile([128, TT], BF16, tag="a", name=f"a1_{it}")
                nc.scalar.activation(a1[:], pg1[:], AF.Tanh, bias=bg1_v)
                b1 = actp.tile([128, TT], BF16, tag="b", name=f"b1_{it}")
                nc.scalar.activation(b1[:], pg1[:], AF.Sigmoid, bias=bg1_v)
                weave(jt, 4, state)
                heartbeat(2)
                g1 = gtp.tile([128, TT], BF16, tag="g1", name=f"g1_{it}")
                nc.vector.tensor_mul(g1[:], a1[:], b1[:])
                weave(jt, 5, state)

                # -- head: s0 = relu(skip + bias), s1 = relu(wsk1@s0 + bias)
                nc.tensor.matmul(psk[:], wg_s[:, 3, :], g1[:],
                                 start=False, stop=True)
                heartbeat(2)
                weave(jt, 6, state)
                s0_t = gtp.tile([128, TT], BF16, tag="s0", name=f"s0_{it}")
                # relu on DVE: ACT is the busier engine (4 transcendentals +
                # 4 evac copies/tile); the chain delay lands in DELTA slack
                nc.vector.tensor_scalar(s0_t[:], psk[:], bskip_v, 0.0,
                                        ALU.add, ALU.max)
                ps5 = pksp.tile([128, TT], FP32, tag="pks", name=f"ps5_{it}")
                # time-deinterleave via the moving operand: ps5 phys col
                # u*128+p gets t=4p+u, so the out-stage chunks produce
                # 4-consecutive-t store partitions at zero engine cost
                s0_deint = s0_t[:].rearrange("f (p u) -> f u p", p=128, u=4)
                nc.tensor.matmul(ps5[:], wg_s[:, 4, :], s0_deint,
                                 start=True, stop=True)
                heartbeat(2)
                weave(jt, 7, state)
                nc.vector.tensor_scalar(s1_s[:, t0:t0 + TT], ps5[:],
                                        bsk1_v, 0.0, ALU.add, ALU.max)

            state = {}
            for it in range(NT):
                emit_body(it, state)
            # trailing: back half of out(NT-2) interleaved with the front
            # half of out(NT-1), then the rest of out(NT-1)
            for k in range(4):
                out_stage(NT - 2, 4 + k, state)
                out_stage(NT - 1, k, state)
                heartbeat(1)
            for k in range(4, 8):
                out_stage(NT - 1, k, state)
                heartbeat(1)

    nc.compile()
    return nc


def get_nc():
    global _cached_nc
    if _cached_nc is None:
        _cached_nc = build_nc()
    return _cached_nc


def kernel(**inputs):
    nc = get_nc()
    w = prepare_weights(
        inputs["w_causal"], inputs["b_causal"],
        inputs["wd0"], inputs["bd0"], inputs["ws0"], inputs["bs0"],
        inputs["wo0"], inputs["bo0"],
        inputs["wd1"], inputs["bd1"], inputs["ws1"], inputs["bs1"],
        inputs["wo1"], inputs["bo1"],
        inputs["w_sk1"], inputs["b_sk1"], inputs["w_sk2"], inputs["b_sk2"])
    x = np.asarray(inputs["x"])
    in_maps = [{"xT": prepare_x(x, c), **w} for c in range(N_CORES)]
    res = run_bass_kernel_spmd(nc, in_maps, list(range(N_CORES)))
    out = np.concatenate(
        [np.asarray(res.results[c]["y"]) for c in range(N_CORES)], axis=0)
    out = out.astype(np.float32) + _f(inputs["b_sk2"])[None, None, :]
    return out


# revision 25
# speedup vs baseline: 1.1957x; 1.0018x over previous
"""Trainium2 Bass kernel for the DiCNN (WaveNet-like) module — v4.

Sharding: pure data parallelism - 4 batch items per core on 8 cores.
On-chip layout: channels on partitions, time on the free dim; the four
batch items are stacked as 4x32-partition bands (block-diag weights),
2x64 bands for the 64-channel input layer.

v4 design notes (vs the 106.6us v2/v3 baseline):
- Conv composition: pg0 = (wd0*wc) (*) x (3 taps) and
  pg1 = (wd1*wc) (*) x (4 taps) + (wd1*wo0) (*)_2 g0 (2 taps), with all
  biases folded in on the host.  This deletes z0/z1 and their PSUM->SBUF
  round-trips from the per-tile critical chain (z0-add -> pg0 and
  psB -> z1 -> pg1 were ~2.3us of serial latency per tile).
- PE HAM clock pinning: the PE defaults to K=4/8 (1.2 GHz) and only
  reaches 2.4 GHz after a ~3.4us fully-busy activity window; the trace
  showed the whole steady state ran at 1.2 GHz.  Dependency-free
  heartbeat matmuls (dedicated PSUM bank) fill every dependency stall
  so the array stays in the fast state.
- Output stores are 3584B-contiguous per partition: s1 is stored
  time-deinterleaved (phys col u*128+p holds t=4p+u) so each store
  partition covers 4 consecutive t rows (DMA descriptors 896B -> 3584B;
  store queue time ~62us -> ~41us aggregate).
- Output stored bf16; final bias b_sk2 and fp32 upcast happen on host.
"""

import numpy as np
import ml_dtypes

import concourse.bacc as bacc
import concourse.tile as tile
from concourse import mybir
from concourse.bass_utils import run_bass_kernel_spmd

BF16 = mybir.dt.bfloat16
FP32 = mybir.dt.float32

B, T, C_IN, HID, C_OUT, K = 32, 4096, 64, 32, 448, 2
N_CORES = 8
BPC = B // N_CORES          # batches per core = 4
TT = 512                    # time-tile size
NT = T // TT                # 8 tiles
XCOLS = 4100                # 3 zero lead cols + 4096 + 1 pad
DELTA = 2                   # output-stage pipeline delay in tiles
N_WARMUP = 14               # dependency-free warm-up matmuls
WARM_COLS = 384             # warm-up heartbeat width
HB_COLS = 256               # steady-state heartbeat width

AF = mybir.ActivationFunctionType
ALU = mybir.AluOpType

_cached_nc = None


def _f(x):
    return np.asarray(x, dtype=np.float32)


def _bf(x):
    return np.asarray(x, dtype=np.float32).astype(ml_dtypes.bfloat16)


def _tile4(v):
    return np.tile(_f(v).reshape(-1), 4)


def diag4(w32):
    s = np.zeros((128, 128), np.float32)
    for i in range(4):
        s[32 * i:32 * i + 32, 32 * i:32 * i + 32] = w32
    return s


def pairmat(w_32x64):
    """[32hid, 64in] -> two [128,128] block mats (one per batch pair)."""
    out = np.zeros((2, 128, 128), np.float32)
    wT = w_32x64.T  # [64, 32]
    for p in range(2):
        s = np.zeros((128, 128), np.float32)
        s[0:64, 64 * p:64 * p + 32] = wT
        s[64:128, 64 * p + 32:64 * p + 64] = wT
        out[p] = s
    return out


def prepare_weights(w_causal, b_causal, wd0, bd0, ws0, bs0, wo0, bo0,
                    wd1, bd1, ws1, bs1, wo1, bo1, w_sk1, b_sk1, w_sk2, b_sk2):
    """Host-side conv composition + weight layout (identical per core)."""
    del wo1, bo1  # dead code: z after the last block is never used

    Wc = [_f(w_causal)[:, :, k] for k in range(2)]      # [32, 64]
    Wd0 = [_f(wd0)[:, :, k] for k in range(2)]          # [32, 32]
    Wd1 = [_f(wd1)[:, :, k] for k in range(2)]
    Wo0 = _f(wo0)[:, :, 0]

    # pg0 taps on x[t-2+m], m in 0..2
    P = [Wd0[0] @ Wc[0],
         Wd0[0] @ Wc[1] + Wd0[1] @ Wc[0],
         Wd0[1] @ Wc[1]]
    # pg1 taps on x[t-3+m], m = 2k+j in 0..3
    Q = [Wd1[k] @ Wc[j] for k in range(2) for j in range(2)]
    # pg1 taps on g0[t-2+2k]
    R = [Wd1[0] @ Wo0, Wd1[1] @ Wo0]

    # x-tap weights: [128, 2 pairs * 7 taps, 128]; slot 7p+m: pg0 m=0..2,
    # pg1 m=3..6
    wx = np.zeros((128, 14, 128), np.float32)
    for p in range(2):
        for m in range(3):
            wx[:, 7 * p + m, :] = pairmat(P[m])[p]
        for m in range(4):
            wx[:, 7 * p + 3 + m, :] = pairmat(Q[m])[p]

    # diag4 32x32 mats: R0, R1, ws0, ws1, w_sk1
    wg = np.zeros((128, 5, 128), np.float32)
    wg[:, 0, :] = diag4(R[0].T)
    wg[:, 1, :] = diag4(R[1].T)
    wg[:, 2, :] = diag4(_f(ws0)[:, :, 0].T)
    wg[:, 3, :] = diag4(_f(ws1)[:, :, 0].T)
    wg[:, 4, :] = diag4(_f(w_sk1)[:, :, 0].T)

    # final conv weights, replicated on all four 32-partition bands
    w2r = np.zeros((128, 448), np.float32)
    w2T = _f(w_sk2)[:, :, 0].T
    for q in range(4):
        w2r[32 * q:32 * q + 32, :] = w2T

    bc = _f(b_causal)
    bg0 = (Wd0[0] + Wd0[1]) @ bc + _f(bd0)
    bg1 = (Wd1[0] + Wd1[1]) @ (bc + _f(bo0)) + _f(bd1)
    bvecs = np.zeros((128, 4), np.float32)
    bvecs[:, 0] = _tile4(bg0)
    bvecs[:, 1] = _tile4(bg1)
    bvecs[:, 2] = _tile4(_f(bs0) + _f(bs1))
    bvecs[:, 3] = _tile4(b_sk1)

    return dict(wx=_bf(wx), wg=_bf(wg), w2r=_bf(w2r),
                bvecs=np.ascontiguousarray(bvecs))


def prepare_x(x, core):
    """Per-core pre-transposed input staging array [2, 128, XCOLS] bf16.

    Cols 0..2 are the causal zero pad (t=-3..-1); col 3+t holds x[b,t,:]
    for the two batches of pair p stacked on the channel axis.
    """
    xT = np.zeros((2, 128, XCOLS), ml_dtypes.bfloat16)
    xb = _bf(x)
    for p in range(2):
        xT[p, 0:64, 3:3 + T] = xb[4 * core + 2 * p].T
        xT[p, 64:128, 3:3 + T] = xb[4 * core + 2 * p + 1].T
    return xT


def build_nc():
    nc = bacc.Bacc("TRN2", target_bir_lowering=False, debug=False,
                   num_devices=N_CORES)

    xT_d = nc.dram_tensor("xT", [2, 128, XCOLS], BF16, kind="ExternalInput")
    wx_d = nc.dram_tensor("wx", [128, 14, 128], BF16, kind="ExternalInput")
    wg_d = nc.dram_tensor("wg", [128, 5, 128], BF16, kind="ExternalInput")
    w2_d = nc.dram_tensor("w2r", [128, 448], BF16, kind="ExternalInput")
    bv_d = nc.dram_tensor("bvecs", [128, 4], FP32, kind="ExternalInput")
    y_d = nc.dram_tensor("y", [BPC, T, C_OUT], BF16, kind="ExternalOutput")

    with tile.TileContext(nc) as tc:
        with (
            tc.tile_pool(name="const", bufs=1) as const,
            tc.tile_pool(name="persist", bufs=1) as persist,
            tc.tile_pool(name="act", bufs=3) as actp,
            tc.tile_pool(name="gtile", bufs=2) as gtp,
            tc.tile_pool(name="stg", bufs=2) as stgp,
            tc.tile_pool(name="pgg", bufs=2, space="PSUM") as pggp,
            tc.tile_pool(name="pks", bufs=1, space="PSUM") as pksp,
            tc.tile_pool(name="pout", bufs=2, space="PSUM") as poutp,
            tc.tile_pool(name="phb", bufs=1, space="PSUM") as phbp,
        ):
            # ---- heartbeat operands first: memset on GpSimd (the earliest
            # engine out of the preamble, and otherwise idle) so the PE can
            # start warming the HAM clock before any input arrives ----
            hbw_s = const.tile([128, 128], BF16)
            nc.gpsimd.memset(hbw_s[:], 0.0)
            wu_t = persist.tile([128, TT], BF16, tag="wu")
            nc.gpsimd.memset(wu_t[:], 0.0)
            hb_ps = phbp.tile([128, TT], FP32, tag="phb", name="hb_ps")

            hb_cnt = [0]

            def heartbeat(n, cols=HB_COLS):
                """Dependency-free PE filler matmuls: keep the HAM activity
                window busy so the 2.4 GHz clock state is reached/held."""
                for _ in range(n):
                    hb_cnt[0] += 1
                    nc.tensor.matmul(hb_ps[:, 0:cols], hbw_s[:],
                                     wu_t[:, 0:cols], start=True, stop=True)

            # ---- constants (wx first - the warm-up burst needs it) ----
            wx_s = const.tile([128, 14, 128], BF16)
            nc.sync.dma_start(wx_s[:], wx_d.ap())
            # x chunk 0 next: it gates tile 0's first conv group
            x_s = [persist.tile([128, XCOLS], BF16, tag=f"x{p}",
                                name=f"x_s{p}") for p in range(2)]
            XCUTS = (0, 1028, 2052, 3076, XCOLS)
            for p in range(2):
                nc.sync.dma_start(x_s[p][:, 0:XCUTS[1]], xT_d[p, :, 0:XCUTS[1]])
            wg_s = const.tile([128, 5, 128], BF16)
            nc.sync.dma_start(wg_s[:], wg_d.ap())
            w2_s = const.tile([128, 448], BF16)
            nc.sync.dma_start(w2_s[:], w2_d.ap())
            bv_s = const.tile([128, 4], FP32)
            nc.sync.dma_start(bv_s[:], bv_d.ap())

            bg0_v = bv_s[:, 0:1]
            bg1_v = bv_s[:, 1:2]
            bskip_v = bv_s[:, 2:3]
            bsk1_v = bv_s[:, 3:4]

            # ---- persistent activations ----
            for a, b in zip(XCUTS[1:], XCUTS[2:]):
                for p in range(2):
                    nc.sync.dma_start(x_s[p][:, a:b], xT_d[p, :, a:b])
            g0_s = persist.tile([128, 4100], BF16, tag="g0")
            nc.vector.memset(g0_s[:, 0:2], 0.0)
            # s1 features, time-deinterleaved within each 512 tile:
            # phys col u*128+p holds t = 4p+u
            s1_s = persist.tile([128, T], BF16, tag="s1")

            # dummy sigmoid first: steer the ACT-table pass to the
            # sigmoid_and_others set (contains tanh/relu/copy too) so only
            # one ACT_TABLE_LOAD is emitted instead of two
            scr_t = const.tile([128, 8], FP32)
            nc.vector.memset(scr_t[:], 0.0)
            nc.scalar.activation(scr_t[:], scr_t[:], AF.Sigmoid)

            # ---- PE warm-up burst (overlaps the input DMAs) ----
            heartbeat(N_WARMUP, cols=WARM_COLS)

            evac_cnt = [0]

            def emit_out_group(it, j, h, po):
                """Final matmuls for chunk j, band pair h of tile it: the two
                bands as a row-tiled pack (distinct row groups -> concurrent
                PE sub-arrays, LDWEIGHTS overlaps in-flight matmuls)."""
                tc0 = TT * it + 128 * j
                for b in (2 * h, 2 * h + 1):
                    nc.tensor.matmul(po[:, b - 2 * h, 0:448],
                                     s1_s[32 * b:32 * b + 32, tc0:tc0 + 128],
                                     w2_s[32 * b:32 * b + 32, :],
                                     start=True, stop=True,
                                     tile_position=(32 * b, 0))

            def emit_evac(it, j, h, po, stg):
                # strict alternation: consecutive pout slots must evacuate
                # on different engines so the slot ping-pong overlaps
                if evac_cnt[0] % 2 == 1:
                    nc.scalar.copy(stg[:, 2 * h:2 * h + 2, j, :],
                                   po[:, :, 0:448])
                else:
                    nc.vector.tensor_copy(stg[:, 2 * h:2 * h + 2, j, :],
                                          po[:, :, 0:448])
                evac_cnt[0] += 1

            def emit_stores(it, stg, qs):
                t0 = TT * it
                for q in qs:
                    yap = y_d[q, t0:t0 + TT, :].rearrange(
                        "(p u) c -> p u c", p=128, u=4)
                    nc.sync.dma_start(yap, stg[:, q])

            def out_stage(it, phase, state):
                """Emit one phase (0..7) of tile `it`'s output stage."""
                if it < 0:
                    if phase % 2 == 0:
                        heartbeat(1)
                    return
                if phase == 0:
                    state[it] = stgp.tile([128, 4, 4, 448], BF16,
                                          tag="stg", name=f"stg_{it}")
                j, h = phase >> 1, phase & 1
                po = poutp.tile([128, 2, TT], FP32, tag="po",
                                name=f"po_{it}_{phase}")
                emit_out_group(it, j, h, po)
                emit_evac(it, j, h, po, state[it])
                if phase == 6:
                    emit_stores(it, state[it], (0, 1))
                elif phase == 7:
                    emit_stores(it, state[it], (2, 3))

            def weave(it, slot, state):
                # half-shifted schedule: body(i) carries the BACK half of
                # out(i-2) then the FRONT half of out(i-1).  s1(i-1) gets a
                # half-tile of slack to clear the DVE queue before its out
                # stage starts, and only 1.5 out-stages trail the last body.
                if slot < 4:
                    out_stage(it - 2, 4 + slot, state)
                else:
                    out_stage(it - 1, slot - 4, state)

            def emit_body(it, state):
                t0 = TT * it
                jt = it

                # -- pg0 = (wd0*wc) (*) x : 3 taps, col-tiled pairs
                pg0 = pggp.tile([128, TT], FP32, tag="pgg", name=f"pg0_{it}")
                for m in range(3):
                    for p in range(2):
                        rhs = x_s[p][:, t0 + 1 + m:t0 + 1 + m + TT]
                        nc.tensor.matmul(pg0[64 * p:64 * p + 64, :],
                                         wx_s[:, 7 * p + m, 64 * p:64 * p + 64],
                                         rhs, start=(m == 0), stop=(m == 2),
                                         tile_position=(0, 64 * p))
                weave(jt, 0, state)
                a0 = actp.tile([128, TT], BF16, tag="a", name=f"a0_{it}")
                nc.scalar.activation(a0[:], pg0[:], AF.Tanh, bias=bg0_v)
                b0 = actp.tile([128, TT], BF16, tag="b", name=f"b0_{it}")
                nc.scalar.activation(b0[:], pg0[:], AF.Sigmoid, bias=bg0_v)
                weave(jt, 1, state)

                # -- pg1 = (wd1*wc) (*) x (4 taps) + (wd1*wo0) (*)_2 g0;
                # x taps first so only the R taps wait on this tile's g0
                pg1 = pggp.tile([128, TT], FP32, tag="pgg", name=f"pg1_{it}")
                for m in range(4):
                    for p in range(2):
                        rhs = x_s[p][:, t0 + m:t0 + m + TT]
                        nc.tensor.matmul(pg1[64 * p:64 * p + 64, :],
                                         wx_s[:, 7 * p + 3 + m,
                                              64 * p:64 * p + 64],
                                         rhs, start=(m == 0), stop=False,
                                         tile_position=(0, 64 * p))
                heartbeat(2)
                nc.vector.tensor_mul(g0_s[:, 2 + t0:2 + t0 + TT], a0[:], b0[:])
                weave(jt, 2, state)
                for k in range(2):
                    nc.tensor.matmul(pg1[:], wg_s[:, k, :],
                                     g0_s[:, t0 + 2 * k:t0 + 2 * k + TT],
                                     start=False, stop=(k == 1))
                # -- skip accum: ws0*g0 now, += ws1*g1 later
                psk = pksp.tile([128, TT], FP32, tag="pks", name=f"psk_{it}")
                nc.tensor.matmul(psk[:], wg_s[:, 2, :],
                                 g0_s[:, 2 + t0:2 + t0 + TT],
                                 start=True, stop=False)
                weave(jt, 3, state)
                a1 = actp.tile([128, TT], BF16, tag="a", name=f"a1_{it}")
                nc.scalar.activation(a1[:], pg1[:], AF.Tanh, bias=bg1_v)
                b1 = actp.tile([128, TT], BF16, tag="b", name=f"b1_{it}")
                nc.scalar.activation(b1[:], pg1[:], AF.Sigmoid, bias=bg1_v)
                weave(jt, 4, state)
                heartbeat(2)
                g1 = gtp.tile([128, TT], BF16, tag="g1", name=f"g1_{it}")
                nc.vector.tensor_mul(g1[:], a1[:], b1[:])
                weave(jt, 5, state)

                # -- head: s0 = relu(skip + bias), s1 = relu(wsk1@s0 + bias)
                nc.tensor.matmul(psk[:], wg_s[:, 3, :], g1[:],
                                 start=False, stop=True)
                heartbeat(2)
                weave(jt, 6, state)
                s0_t = gtp.tile([128, TT], BF16, tag="s0", name=f"s0_{it}")
                # relu on DVE: ACT is the busier engine (4 transcendentals +
                # 4 evac copies/tile); the chain delay lands in DELTA slack
                nc.vector.tensor_scalar(s0_t[:], psk[:], bskip_v, 0.0,
                                        ALU.add, ALU.max)
                ps5 = pksp.tile([128, TT], FP32, tag="pks", name=f"ps5_{it}")
                # time-deinterleave via the moving operand: ps5 phys col
                # u*128+p gets t=4p+u, so the out-stage chunks produce
                # 4-consecutive-t store partitions at zero engine cost
                s0_deint = s0_t[:].rearrange("f (p u) -> f u p", p=128, u=4)
                nc.tensor.matmul(ps5[:], wg_s[:, 4, :], s0_deint,
                                 start=True, stop=True)
                heartbeat(2)
                weave(jt, 7, state)
                nc.vector.tensor_scalar(s1_s[:, t0:t0 + TT], ps5[:],
                                        bsk1_v, 0.0, ALU.add, ALU.max)

            state = {}
            for it in range(NT):
                emit_body(it, state)
            # trailing: back half of out(NT-2) interleaved with the front
            # half of out(NT-1), then the rest of out(NT-1)
            for k in range(4):
                out_stage(NT - 2, 4 + k, state)
                out_stage(NT - 1, k, state)
                heartbeat(1)
            for k in range(4, 8):
                out_stage(NT - 1, k, state)
                heartbeat(1)

    nc.compile()
    return nc


def get_nc():
    global _cached_nc
    if _cached_nc is None:
        _cached_nc = build_nc()
    return _cached_nc


def kernel(**inputs):
    nc = get_nc()
    w = prepare_weights(
        inputs["w_causal"], inputs["b_causal"],
        inputs["wd0"], inputs["bd0"], inputs["ws0"], inputs["bs0"],
        inputs["wo0"], inputs["bo0"],
        inputs["wd1"], inputs["bd1"], inputs["ws1"], inputs["bs1"],
        inputs["wo1"], inputs["bo1"],
        inputs["w_sk1"], inputs["b_sk1"], inputs["w_sk2"], inputs["b_sk2"])
    x = np.asarray(inputs["x"])
    in_maps = [{"xT": prepare_x(x, c), **w} for c in range(N_CORES)]
    res = run_bass_kernel_spmd(nc, in_maps, list(range(N_CORES)))
    out = np.concatenate(
        [np.asarray(res.results[c]["y"]) for c in range(N_CORES)], axis=0)
    out = out.astype(np.float32) + _f(inputs["b_sk2"])[None, None, :]
    return out
